# revision 2
# baseline (speedup 1.0000x reference)
"""AbilityEncoder TRN2 kernel: 3-level tree GNN over 32768 trees.

Strategy: data-parallel over 8 NeuronCores (4096 trees each). On host we
fold the tiny embedding tables through the MLP weight matrices (constant
folding) and encode the integer ids as multi-hot bf16 columns; on device
each node's pre-activation is one PE matmul (multi-hot gather), the
leaf/node select is baked into the one-hot rows (-BIG row forces relu->0),
children are summed and projected with PSUM-accumulated matmuls.
"""
import numpy as np
import ml_dtypes
import sys, os

sys.path.insert(0, "/opt/trn_rl_repo")

H = 96
NODES = 21
N = 32768
NCORES = 8
NPC = N // NCORES          # trees per core
T = 512                    # trees per tile
NT = NPC // T              # tiles per core
BIG = 1.0e4

# one-hot row layout for the A (main) table:  [70 rows]
#  0:7    trig @ W1bot      (node path)
#  7:16   eff  @ W1bot
# 16:26   targ @ W1bot
# 26:41   op   @ W1top
# 41:48   trig @ Wl         (leaf path)
# 48:57   eff  @ Wl
# 57:67   targ @ Wl
# 67      b1 row
# 68      bl row
# 69      b2@W1bot row (leaf-count term, only used when b2 != 0)
RA = 70
# B table (leaf extractor): [28 rows]
#  0:7 trig@Wl  7:16 eff@Wl  16:26 targ@Wl  26 bl row  27 -BIG row
RB = 28


def _build_host_tensors(trigger_ids, action_ids, target_ids, operand_ids,
                        trig_table, eff_table, targ_table, op_table,
                        W1, b1, W2, b2, Wl, bl):
    f64 = np.float64
    W1 = W1.astype(f64); W2 = W2.astype(f64); Wl = Wl.astype(f64)
    W1top, W1bot = W1[:H], W1[H:]
    W2W1 = W2 @ W1bot                      # projects h (node) to parent pre-act
    DD = Wl_side = None
    D = -W2W1.copy()
    D += np.eye(H) @ W1bot                 # (W1bot - W2W1), applied to leaf h
    tblA = np.zeros((RA, H), f64)
    tblA[0:7] = trig_table.astype(f64) @ W1bot
    tblA[7:16] = eff_table.astype(f64) @ W1bot
    tblA[16:26] = targ_table.astype(f64) @ W1bot
    tblA[26:41] = op_table.astype(f64) @ W1top
    tblA[41:48] = trig_table.astype(f64) @ Wl
    tblA[48:57] = eff_table.astype(f64) @ Wl
    tblA[57:67] = targ_table.astype(f64) @ Wl
    tblA[67] = b1.astype(f64)
    tblA[68] = bl.astype(f64)
    tblA[69] = b2.astype(f64) @ W1bot
    tblB = np.zeros((RB, H), f64)
    tblB[0:7] = trig_table.astype(f64) @ Wl
    tblB[7:16] = eff_table.astype(f64) @ Wl
    tblB[16:26] = targ_table.astype(f64) @ Wl
    tblB[26] = bl.astype(f64)
    tblB[27] = -BIG

    # ---- multi-hot encodings ----
    t = trigger_ids.astype(np.int64); a = action_ids.astype(np.int64)
    g = target_ids.astype(np.int64); o = operand_ids.astype(np.int64)
    leaf = (o == 0)

    # A encoding: [RA, N*NODES], column col = n*N + tree
    bf = ml_dtypes.bfloat16
    ohA = np.zeros((RA, NODES * N), bf)
    cols = np.arange(N)

    def setA(n, rows_idx, val=1.0):
        ohA[rows_idx, n * N + cols] = val

    for n in range(NODES):
        if n < 5:
            setA(n, t[:, n]); setA(n, 7 + a[:, n]); setA(n, 16 + g[:, n])
            setA(n, 26 + o[:, n]); setA(n, np.full(N, 67))
        else:
            lf = leaf[:, n]
            nrows = np.where(lf, 41 + t[:, n], t[:, n])
            setA(n, nrows)
            setA(n, np.where(lf, 48 + a[:, n], 7 + a[:, n]))
            setA(n, np.where(lf, 57 + g[:, n], 16 + g[:, n]))
            # op row: leaf has o==0 and op_table[0] @ W1top is NOT zero in
            # general... but reference uses op_emb only on node path; for
            # leaves op_table[o=0] row IS the padding zero row, so adding
            # row 26+0 is exact 0 contribution. Add it only for non-leaf.
            onr = 26 + o[:, n]
            ohA[onr[~lf], n * N + cols[~lf]] = 1.0
            setA(n, np.where(lf, 68, 67))
            if np.any(b2 != 0):
                # leaf-count term: parent gets b2@W1bot per NON-leaf child
                ohA[69, n * N + cols[~lf]] = ohA[69, n * N + cols[~lf]] + 1.0

    # B encoding: [RB, 16*N], column col = (n-5)*N + tree ; h_leaf extractor
    ohB = np.zeros((RB, 16 * N), bf)
    for n in range(5, NODES):
        k = n - 5
        lf = leaf[:, n]
        ohB[t[lf, n], k * N + cols[lf]] = 1.0
        ohB[7 + a[lf, n], k * N + cols[lf]] = 1.0
        ohB[16 + g[lf, n], k * N + cols[lf]] = 1.0
        ohB[26, k * N + cols[lf]] = 1.0
        ohB[27, k * N + cols[~lf]] = 1.0   # -BIG -> relu gives 0

    W2aug = np.zeros((H + 1, H), f64)
    W2aug[:H] = W2
    W2aug[H] = b2.astype(f64)
    b2w1x4 = 4.0 * (b2.astype(f64) @ W1bot)

    return (ohA, ohB,
            tblA.astype(bf), tblB.astype(bf),
            W2W1.astype(bf), D.astype(bf), W2aug.astype(bf),
            b2w1x4.astype(np.float32).reshape(H, 1))


_CACHED = {}


def _build_program():
    import concourse.bass as bass
    import concourse.tile as tile
    import concourse.mybir as mybir
    from concourse.vector_clock import ScopedClock
    import bass_rust as _br

    # --- patch: this walrus accepts only 1 sem wait per instruction ---
    def _drain_and_barrier(self, tick_clock, wait_clock):
        nc_ = self.nc
        probe = nc_.sync.drain()
        wait_clock.add_sem_waits(probe.ins,
                                 ScopedClock({None: tick_clock.global_clock}))
        si = probe.ins.sync_info
        waits = list(si.on_wait) if si is not None else []
        if len(waits) > 1:
            si.on_wait = waits[:1]
            for w in waits[1:]:
                extra = nc_.sync.drain()
                extra.ins.sync_info = _br.SyncInfo(on_wait=[w], on_update=[])
        nc_.all_engine_barrier()
        popped = nc_._tile_sem_poison_stack.pop()
        assert popped is self._sem_poison
        nc_.clear_and_free_semaphores(list(self.sems.allocated().values()))
        nc_.all_engine_barrier()

    tile.TileContext._drain_and_barrier = _drain_and_barrier

    def split_waits(nc_, max_waits=1):
        import concourse.mybir as mybir_
        for f in nc_.m.functions:
            for bb in f.blocks:
                out = []
                for inst in bb.instructions:
                    si = inst.sync_info
                    if si is not None:
                        waits = list(si.on_wait)
                        if len(waits) > max_waits:
                            extra, keep = waits[:-max_waits], waits[-max_waits:]
                            for j, w in enumerate(extra):
                                ev = mybir_.InstEventSemaphore(
                                    name=f"{inst.name}-xw{j}")
                                ev.engine = inst.engine
                                ev.sync_info = _br.SyncInfo(
                                    on_wait=[w], on_update=[])
                                out.append(ev)
                            si.on_wait = keep
                    out.append(inst)
                try:
                    bb.instructions = out
                except Exception:
                    bb.instructions.clear()
                    for i_ in out:
                        bb.instructions.append(i_)

    dt = mybir.dt
    nc = bass.Bass(trn_type="TRN2", target_bir_lowering=False, debug=False)
    ohA_d = nc.dram_tensor("ohA", [RA, NODES * NPC], dt.bfloat16,
                           kind="ExternalInput").ap()
    ohB_d = nc.dram_tensor("ohB", [RB, 16 * NPC], dt.bfloat16,
                           kind="ExternalInput").ap()
    tblA_d = nc.dram_tensor("tblA", [RA, H], dt.bfloat16,
                            kind="ExternalInput").ap()
    tblB_d = nc.dram_tensor("tblB", [RB, H], dt.bfloat16,
                            kind="ExternalInput").ap()
    w2w1_d = nc.dram_tensor("w2w1", [H, H], dt.bfloat16,
                            kind="ExternalInput").ap()
    dmat_d = nc.dram_tensor("dmat", [H, H], dt.bfloat16,
                            kind="ExternalInput").ap()
    w2aug_d = nc.dram_tensor("w2aug", [H + 1, H], dt.bfloat16,
                             kind="ExternalInput").ap()
    b2c_d = nc.dram_tensor("b2c", [H, 1], dt.float32,
                           kind="ExternalInput").ap()
    out_d = nc.dram_tensor("out", [NPC, H], dt.float32,
                           kind="ExternalOutput").ap()

    with tile.TileContext(nc) as tc:
        with tc.tile_pool(name="const", bufs=1) as cpool, \
             tc.tile_pool(name="oh", bufs=3) as ohpool, \
             tc.tile_pool(name="hbuf", bufs=3) as hpool, \
             tc.tile_pool(name="sums", bufs=2) as spool, \
             tc.tile_pool(name="psA", bufs=2, space="PSUM") as ppA, \
             tc.tile_pool(name="ps1", bufs=1, space="PSUM") as pp1, \
             tc.tile_pool(name="pso", bufs=2, space="PSUM") as ppo:

            tblA_s = cpool.tile([RA, H], dt.bfloat16)
            nc.sync.dma_start(tblA_s[:], tblA_d[:])
            tblB_s = cpool.tile([RB, H], dt.bfloat16)
            nc.sync.dma_start(tblB_s[:], tblB_d[:])
            w2w1_s = cpool.tile([H, H], dt.bfloat16)
            nc.sync.dma_start(w2w1_s[:], w2w1_d[:])
            dmat_s = cpool.tile([H, H], dt.bfloat16)
            nc.sync.dma_start(dmat_s[:], dmat_d[:])
            w2aug_s = cpool.tile([H + 1, H], dt.bfloat16)
            nc.sync.dma_start(w2aug_s[:], w2aug_d[:])
            b2c_s = cpool.tile([H, 1], dt.float32)
            nc.sync.dma_start(b2c_s[:], b2c_d[:])

            for it in range(NT):
                c0 = it * T
                # ---- level 2: per node gather -> h / h_leaf ----
                hs = []
                hls = []
                for k in range(16):
                    n = 5 + k
                    ohA_s = ohpool.tile([RA, T], dt.bfloat16, tag="ohA")
                    nc.sync.dma_start(ohA_s[:],
                                      ohA_d[:, n * NPC + c0:n * NPC + c0 + T])
                    psA = ppA.tile([H, T], dt.float32, tag="psA")
                    nc.tensor.matmul(psA[:], tblA_s[:], ohA_s[:],
                                     start=True, stop=True)
                    h = hpool.tile([H, T], dt.bfloat16, tag="h")
                    # alternate relu-drain between ACT and DVE
                    if k % 2 == 0:
                        nc.scalar.activation(h[:], psA[:],
                                             mybir.ActivationFunctionType.Relu)
                    else:
                        nc.vector.tensor_scalar_max(h[:], psA[:], 0.0)
                    hs.append(h)

                    ohB_s = ohpool.tile([RB, T], dt.bfloat16, tag="ohB")
                    nc.sync.dma_start(ohB_s[:],
                                      ohB_d[:, k * NPC + c0:k * NPC + c0 + T])
                    psB = ppA.tile([H, T], dt.float32, tag="psB")
                    nc.tensor.matmul(psB[:], tblB_s[:], ohB_s[:],
                                     start=True, stop=True)
                    hl = hpool.tile([H, T], dt.bfloat16, tag="hl")
                    if k % 2 == 1:
                        nc.scalar.activation(hl[:], psB[:],
                                             mybir.ActivationFunctionType.Relu)
                    else:
                        nc.vector.tensor_scalar_max(hl[:], psB[:], 0.0)
                    hls.append(hl)

                # ---- level 1: sibling sums + projections (PSUM-fused) ----
                h1s = []
                for j in range(4):
                    sh = spool.tile([H, T], dt.bfloat16, tag="sh")
                    nc.gpsimd.tensor_tensor(out=sh[:], in0=hs[4 * j][:],
                                            in1=hs[4 * j + 1][:],
                                            op=mybir.AluOpType.add)
                    nc.gpsimd.tensor_tensor(out=sh[:], in0=sh[:],
                                            in1=hs[4 * j + 2][:],
                                            op=mybir.AluOpType.add)
                    nc.gpsimd.tensor_tensor(out=sh[:], in0=sh[:],
                                            in1=hs[4 * j + 3][:],
                                            op=mybir.AluOpType.add)
                    sl = spool.tile([H, T], dt.bfloat16, tag="sl")
                    nc.vector.tensor_tensor(out=sl[:], in0=hls[4 * j][:],
                                            in1=hls[4 * j + 1][:],
                                            op=mybir.AluOpType.add)
                    nc.vector.tensor_tensor(out=sl[:], in0=sl[:],
                                            in1=hls[4 * j + 2][:],
                                            op=mybir.AluOpType.add)
                    nc.vector.tensor_tensor(out=sl[:], in0=sl[:],
                                            in1=hls[4 * j + 3][:],
                                            op=mybir.AluOpType.add)
                    # pre1_j = Sh@W2W1 + Sl@(W1bot-W2W1) + gatherA(node 1+j)
                    ohA_s = ohpool.tile([RA, T], dt.bfloat16, tag="ohA")
                    nc.sync.dma_start(
                        ohA_s[:],
                        ohA_d[:, (1 + j) * NPC + c0:(1 + j) * NPC + c0 + T])
                    ps1 = pp1.tile([H, T], dt.float32, tag="ps1")
                    nc.tensor.matmul(ps1[:], w2w1_s[:], sh[:],
                                     start=True, stop=False)
                    nc.tensor.matmul(ps1[:], dmat_s[:], sl[:],
                                     start=False, stop=False)
                    nc.tensor.matmul(ps1[:], tblA_s[:], ohA_s[:],
                                     start=False, stop=True)
                    h1 = hpool.tile([H, T], dt.bfloat16, tag="h1")
                    if j % 2 == 0:
                        nc.scalar.activation(h1[:], ps1[:],
                                             mybir.ActivationFunctionType.Relu)
                    else:
                        nc.vector.tensor_scalar_max(h1[:], ps1[:], 0.0)
                    h1s.append(h1)

                # ---- root ----
                s_all = spool.tile([H, T], dt.bfloat16, tag="sroot")
                nc.vector.tensor_tensor(out=s_all[:], in0=h1s[0][:],
                                        in1=h1s[1][:], op=mybir.AluOpType.add)
                nc.gpsimd.tensor_tensor(out=s_all[:], in0=s_all[:],
                                        in1=h1s[2][:], op=mybir.AluOpType.add)
                nc.vector.tensor_tensor(out=s_all[:], in0=s_all[:],
                                        in1=h1s[3][:], op=mybir.AluOpType.add)
                ohA_s = ohpool.tile([RA, T], dt.bfloat16, tag="ohA")
                nc.sync.dma_start(ohA_s[:], ohA_d[:, c0:c0 + T])
                ps0 = pp1.tile([H, T], dt.float32, tag="ps0")
                nc.tensor.matmul(ps0[:], w2w1_s[:], s_all[:],
                                 start=True, stop=False)
                nc.tensor.matmul(ps0[:], tblA_s[:], ohA_s[:],
                                 start=False, stop=True)
                h0aug = hpool.tile([H + 1, T], dt.bfloat16, tag="h0")
                nc.scalar.activation(h0aug[0:H, :], ps0[:],
                                     mybir.ActivationFunctionType.Relu,
                                     bias=b2c_s[:])
                nc.vector.memset(h0aug[H:H + 1, :], 1.0)

                # ---- out = h0.T @ W2 + b2, tree-major ----
                for ch in range(T // 128):
                    pso = ppo.tile([128, H], dt.float32, tag="pso")
                    nc.tensor.matmul(
                        pso[:], h0aug[:, ch * 128:(ch + 1) * 128], w2aug_s[:],
                        start=True, stop=True)
                    osb = hpool.tile([128, H], dt.float32, tag="osb")
                    nc.vector.tensor_copy(out=osb[:], in_=pso[:])
                    nc.sync.dma_start(
                        out_d[c0 + ch * 128:c0 + (ch + 1) * 128, :], osb[:])

    split_waits(nc)
    return nc


def kernel(**inputs) -> np.ndarray:
    from concourse.bass_utils import run_bass_kernel_spmd

    (ohA, ohB, tblA, tblB, W2W1, D, W2aug, b2c) = _build_host_tensors(**inputs)

    if "nc" not in _CACHED:
        _CACHED["nc"] = _build_program()
    nc = _CACHED["nc"]

    bf = ml_dtypes.bfloat16
    in_maps = []
    ohA_nodes = ohA.reshape(RA, NODES, N)
    ohB_nodes = ohB.reshape(RB, 16, N)
    for c in range(NCORES):
        s = slice(c * NPC, (c + 1) * NPC)
        in_maps.append({
            "ohA": np.ascontiguousarray(
                ohA_nodes[:, :, s].reshape(RA, NODES * NPC)),
            "ohB": np.ascontiguousarray(
                ohB_nodes[:, :, s].reshape(RB, 16 * NPC)),
            "tblA": tblA, "tblB": tblB, "w2w1": W2W1, "dmat": D,
            "w2aug": W2aug, "b2c": b2c,
        })
    res = run_bass_kernel_spmd(nc, in_maps, list(range(NCORES)))
    out = np.concatenate([res.results[c]["out"] for c in range(NCORES)],
                         axis=0)
    return out.astype(np.float32)


# revision 5
# speedup vs baseline: 1.0827x; 1.0827x over previous
"""AbilityEncoder TRN2 kernel: 3-level tree GNN over 32768 trees.

Data-parallel over 8 NeuronCores (4096 trees each). Host folds the tiny
embedding tables through the MLP weights (constant folding) and encodes
ids as multi-hot bf16 columns; device computes each node pre-activation
as one PE matmul (multi-hot gather). Leaf/node select is baked into the
one-hot rows (leaf rows address Wl-projected table blocks; a -BIG row
forces relu->0 on the extractor path). Level-1 pre-activations are
PSUM-accumulated: W2W1 @ sum(h_children) + (W1bot-W2W1) @ sum(h_leaf)
+ own gather. The output matmul uses h0 as the stationary operand so the
result lands tree-major for a contiguous DMA out.
"""
import numpy as np
import ml_dtypes
import sys

sys.path.insert(0, "/opt/trn_rl_repo")

H = 96
NODES = 21
N = 32768
NCORES = 8
NPC = N // NCORES          # trees per core
T = 512                    # trees per tile
NT = NPC // T              # tiles per core
BIG = 1.0e4

# A table rows: 0:7 trig@W1bot(+b1) | 7:16 eff@W1bot | 16:26 targ@W1bot |
# 26:41 op@W1top | 41:48 trig@Wl(+bl) | 48:57 eff@Wl | 57:67 targ@Wl |
# 67 b2@W1bot (leaf-count term, only used when b2 != 0)
RA = 68
# B table (leaf-h extractor): 0:7 trig@Wl(+bl) | 7:16 eff@Wl |
# 16:26 targ@Wl | 27 -BIG
RB = 28


def _build_host_tensors(trigger_ids, action_ids, target_ids, operand_ids,
                        trig_table, eff_table, targ_table, op_table,
                        W1, b1, W2, b2, Wl, bl):
    f64 = np.float64
    W1 = W1.astype(f64); W2 = W2.astype(f64); Wl = Wl.astype(f64)
    W1top, W1bot = W1[:H], W1[H:]
    W2W1 = W2 @ W1bot
    D = W1bot - W2W1
    tblA = np.zeros((RA, H), f64)
    tblA[0:7] = trig_table.astype(f64) @ W1bot + b1.astype(f64)
    tblA[7:16] = eff_table.astype(f64) @ W1bot
    tblA[16:26] = targ_table.astype(f64) @ W1bot
    tblA[26:41] = op_table.astype(f64) @ W1top
    tblA[41:48] = trig_table.astype(f64) @ Wl + bl.astype(f64)
    tblA[48:57] = eff_table.astype(f64) @ Wl
    tblA[57:67] = targ_table.astype(f64) @ Wl
    tblA[67] = b2.astype(f64) @ W1bot
    tblB = np.zeros((RB, H), f64)
    tblB[0:7] = trig_table.astype(f64) @ Wl + bl.astype(f64)
    tblB[7:16] = eff_table.astype(f64) @ Wl
    tblB[16:26] = targ_table.astype(f64) @ Wl
    tblB[27] = -BIG

    t = trigger_ids.astype(np.int64); a = action_ids.astype(np.int64)
    g = target_ids.astype(np.int64); o = operand_ids.astype(np.int64)
    leaf = (o == 0)
    bf = ml_dtypes.bfloat16
    cols = np.arange(N)

    # A encoding: [RA, NODES, N]
    ohA = np.zeros((RA, NODES, N), bf)
    for n in range(NODES):
        if n < 5:
            ohA[:, n][t[:, n], cols] = 1.0
            ohA[:, n][7 + a[:, n], cols] = 1.0
            ohA[:, n][16 + g[:, n], cols] = 1.0
            ohA[:, n][26 + o[:, n], cols] = 1.0
        else:
            lf = leaf[:, n]
            ohA[:, n][np.where(lf, 41 + t[:, n], t[:, n]), cols] = 1.0
            ohA[:, n][np.where(lf, 48 + a[:, n], 7 + a[:, n]), cols] = 1.0
            ohA[:, n][np.where(lf, 57 + g[:, n], 16 + g[:, n]), cols] = 1.0
            onr = 26 + o[:, n]
            ohA[:, n][onr[~lf], cols[~lf]] = 1.0
            if np.any(b2 != 0):
                ohA[:, n][67, cols[~lf]] = 1.0

    # B encoding: [RB, 16, N] (level-2 nodes only)
    ohB = np.zeros((RB, 16, N), bf)
    for n in range(5, NODES):
        k = n - 5
        lf = leaf[:, n]
        ohB[:, k][t[lf, n], cols[lf]] = 1.0
        ohB[:, k][7 + a[lf, n], cols[lf]] = 1.0
        ohB[:, k][16 + g[lf, n], cols[lf]] = 1.0
        ohB[:, k][27, cols[~lf]] = 1.0

    W2aug = np.zeros((H + 1, H), f64)
    W2aug[:H] = W2
    W2aug[H] = b2.astype(f64)
    b2w1x4 = 4.0 * (b2.astype(f64) @ W1bot)

    return (ohA, ohB, tblA.astype(bf), tblB.astype(bf),
            W2W1.astype(bf), D.astype(bf), W2aug.astype(bf),
            b2w1x4.astype(np.float32).reshape(H, 1))


_CACHED = {}


def _build_program():
    import concourse.bass as bass
    import concourse.tile as tile
    import concourse.mybir as mybir
    from concourse.vector_clock import ScopedClock
    import bass_rust as _br

    # --- this walrus accepts only 1 sem wait per instruction: patch the
    # tile tail drain and post-split all other instructions ---
    def _drain_and_barrier(self, tick_clock, wait_clock):
        nc_ = self.nc
        probe = nc_.sync.drain()
        wait_clock.add_sem_waits(probe.ins,
                                 ScopedClock({None: tick_clock.global_clock}))
        si = probe.ins.sync_info
        waits = list(si.on_wait) if si is not None else []
        if len(waits) > 1:
            si.on_wait = waits[:1]
            for w in waits[1:]:
                extra = nc_.sync.drain()
                extra.ins.sync_info = _br.SyncInfo(on_wait=[w], on_update=[])
        nc_.all_engine_barrier()
        popped = nc_._tile_sem_poison_stack.pop()
        assert popped is self._sem_poison
        nc_.clear_and_free_semaphores(list(self.sems.allocated().values()))
        nc_.all_engine_barrier()

    tile.TileContext._drain_and_barrier = _drain_and_barrier

    def split_waits(nc_, max_waits=1):
        for f in nc_.m.functions:
            for bb in f.blocks:
                out = []
                for inst in bb.instructions:
                    si = inst.sync_info
                    if si is not None:
                        waits = list(si.on_wait)
                        if len(waits) > max_waits:
                            extra, keep = waits[:-max_waits], waits[-max_waits:]
                            for j, w in enumerate(extra):
                                ev = mybir.InstEventSemaphore(
                                    name=f"{inst.name}-xw{j}")
                                ev.engine = inst.engine
                                ev.sync_info = _br.SyncInfo(
                                    on_wait=[w], on_update=[])
                                out.append(ev)
                            si.on_wait = keep
                    out.append(inst)
                try:
                    bb.instructions = out
                except Exception:
                    bb.instructions.clear()
                    for i_ in out:
                        bb.instructions.append(i_)

    dt = mybir.dt
    Relu = mybir.ActivationFunctionType.Relu
    ADD = mybir.AluOpType.add
    nc = bass.Bass(trn_type="TRN2", target_bir_lowering=False, debug=False)
    ohA_d = nc.dram_tensor("ohA", [RA, NT * NODES * T], dt.bfloat16,
                           kind="ExternalInput").ap()
    ohB_d = nc.dram_tensor("ohB", [RB, NT * 16 * T], dt.bfloat16,
                           kind="ExternalInput").ap()
    tblA_d = nc.dram_tensor("tblA", [RA, H], dt.bfloat16,
                            kind="ExternalInput").ap()
    tblB_d = nc.dram_tensor("tblB", [RB, H], dt.bfloat16,
                            kind="ExternalInput").ap()
    w2w1_d = nc.dram_tensor("w2w1", [H, H], dt.bfloat16,
                            kind="ExternalInput").ap()
    dmat_d = nc.dram_tensor("dmat", [H, H], dt.bfloat16,
                            kind="ExternalInput").ap()
    w2aug_d = nc.dram_tensor("w2aug", [H + 1, H], dt.bfloat16,
                             kind="ExternalInput").ap()
    b2c_d = nc.dram_tensor("b2c", [H, 1], dt.float32,
                           kind="ExternalInput").ap()
    out_d = nc.dram_tensor("out", [NPC, H], dt.float32,
                           kind="ExternalOutput").ap()

    with tile.TileContext(nc) as tc:
        with tc.tile_pool(name="const", bufs=1) as cpool, \
             tc.tile_pool(name="oh", bufs=2) as ohpool, \
             tc.tile_pool(name="hbuf", bufs=2) as hpool, \
             tc.tile_pool(name="ps", bufs=4, space="PSUM") as pspool:

            tblA_s = cpool.tile([RA, H], dt.bfloat16)
            nc.sync.dma_start(tblA_s[:], tblA_d[:])
            tblB_s = cpool.tile([RB, H], dt.bfloat16)
            nc.sync.dma_start(tblB_s[:], tblB_d[:])
            w2w1_s = cpool.tile([H, H], dt.bfloat16)
            nc.sync.dma_start(w2w1_s[:], w2w1_d[:])
            dmat_s = cpool.tile([H, H], dt.bfloat16)
            nc.sync.dma_start(dmat_s[:], dmat_d[:])
            w2aug_s = cpool.tile([H + 1, H], dt.bfloat16)
            nc.sync.dma_start(w2aug_s[:], w2aug_d[:])
            b2c_s = cpool.tile([H, 1], dt.float32)
            nc.sync.dma_start(b2c_s[:], b2c_d[:])

            for it in range(NT):
                ohA_s = ohpool.tile([RA, NODES * T], dt.bfloat16, tag="ohA")
                nc.sync.dma_start(
                    ohA_s[:], ohA_d[:, it * NODES * T:(it + 1) * NODES * T])
                ohB_s = ohpool.tile([RB, 16 * T], dt.bfloat16, tag="ohB")
                nc.sync.dma_start(
                    ohB_s[:], ohB_d[:, it * 16 * T:(it + 1) * 16 * T])

                # ---- level 2 gathers: h (A path) and h_leaf (B path) ----
                h = hpool.tile([H, 16 * T], dt.bfloat16, tag="h")
                hl = hpool.tile([H, 16 * T], dt.bfloat16, tag="hl")
                for i in range(8):
                    c0 = i * 2 * T
                    psA = pspool.tile([128, 2 * T], dt.float32, tag="g")
                    nc.tensor.matmul(psA[0:H, 0:T], tblA_s[:],
                                     ohA_s[:, 5 * T + c0:5 * T + c0 + T],
                                     start=True, stop=True)
                    nc.tensor.matmul(psA[0:H, T:2 * T], tblA_s[:],
                                     ohA_s[:, 5 * T + c0 + T:5 * T + c0 + 2 * T],
                                     start=True, stop=True)
                    if i % 2 == 0:
                        nc.scalar.activation(h[:, c0:c0 + 2 * T],
                                             psA[0:H, :], Relu)
                    else:
                        nc.vector.tensor_scalar_max(h[:, c0:c0 + 2 * T],
                                                    psA[0:H, :], 0.0)
                    psB = pspool.tile([128, 2 * T], dt.float32, tag="g")
                    nc.tensor.matmul(psB[0:H, 0:T], tblB_s[:],
                                     ohB_s[:, c0:c0 + T],
                                     start=True, stop=True)
                    nc.tensor.matmul(psB[0:H, T:2 * T], tblB_s[:],
                                     ohB_s[:, c0 + T:c0 + 2 * T],
                                     start=True, stop=True)
                    if i % 2 == 1:
                        nc.scalar.activation(hl[:, c0:c0 + 2 * T],
                                             psB[0:H, :], Relu)
                    else:
                        nc.vector.tensor_scalar_max(hl[:, c0:c0 + 2 * T],
                                                    psB[0:H, :], 0.0)

                # ---- sibling sums: h is (j, c, T); sum over c ----
                sh = hpool.tile([H, 4 * T], dt.bfloat16, tag="sh")
                h4 = h[:].rearrange("p (j c t) -> p j c t", j=4, c=4)
                sh4 = sh[:].rearrange("p (j t) -> p j t", j=4)
                nc.vector.tensor_tensor(out=sh4, in0=h4[:, :, 0, :],
                                        in1=h4[:, :, 1, :], op=ADD)
                nc.vector.tensor_tensor(out=sh4, in0=sh4,
                                        in1=h4[:, :, 2, :], op=ADD)
                nc.vector.tensor_tensor(out=sh4, in0=sh4,
                                        in1=h4[:, :, 3, :], op=ADD)
                sl = hpool.tile([H, 4 * T], dt.bfloat16, tag="sl")
                hl4 = hl[:].rearrange("p (j c t) -> p j c t", j=4, c=4)
                sl4 = sl[:].rearrange("p (j t) -> p j t", j=4)
                nc.vector.tensor_tensor(out=sl4, in0=hl4[:, :, 0, :],
                                        in1=hl4[:, :, 1, :], op=ADD)
                nc.vector.tensor_tensor(out=sl4, in0=sl4,
                                        in1=hl4[:, :, 2, :], op=ADD)
                nc.vector.tensor_tensor(out=sl4, in0=sl4,
                                        in1=hl4[:, :, 3, :], op=ADD)

                # ---- level 1: pre1_j = W2W1@Sh_j + D@Sl_j + gather(1+j) ----
                h1 = hpool.tile([H, 4 * T], dt.bfloat16, tag="h1")
                for half in range(2):
                    ps1 = pspool.tile([128, 2 * T], dt.float32, tag="g")
                    for q in range(2):
                        j = 2 * half + q
                        o_ = q * T
                        nc.tensor.matmul(ps1[0:H, o_:o_ + T], w2w1_s[:],
                                         sh[:, j * T:(j + 1) * T],
                                         start=True, stop=False)
                        nc.tensor.matmul(ps1[0:H, o_:o_ + T], dmat_s[:],
                                         sl[:, j * T:(j + 1) * T],
                                         start=False, stop=False)
                        nc.tensor.matmul(ps1[0:H, o_:o_ + T], tblA_s[:],
                                         ohA_s[:, (1 + j) * T:(2 + j) * T],
                                         start=False, stop=True)
                    if half == 0:
                        nc.scalar.activation(h1[:, 0:2 * T], ps1[0:H, :], Relu)
                    else:
                        nc.vector.tensor_scalar_max(h1[:, 2 * T:4 * T],
                                                    ps1[0:H, :], 0.0)

                # ---- root ----
                s0 = hpool.tile([H, T], dt.bfloat16, tag="s0")
                nc.vector.tensor_tensor(out=s0[:], in0=h1[:, 0:T],
                                        in1=h1[:, T:2 * T], op=ADD)
                nc.vector.tensor_tensor(out=s0[:], in0=s0[:],
                                        in1=h1[:, 2 * T:3 * T], op=ADD)
                nc.vector.tensor_tensor(out=s0[:], in0=s0[:],
                                        in1=h1[:, 3 * T:4 * T], op=ADD)
                ps0 = pspool.tile([128, 2 * T], dt.float32, tag="g")
                nc.tensor.matmul(ps0[0:H, 0:T], w2w1_s[:], s0[:],
                                 start=True, stop=False)
                nc.tensor.matmul(ps0[0:H, 0:T], tblA_s[:], ohA_s[:, 0:T],
                                 start=False, stop=True)
                h0 = hpool.tile([H + 1, T], dt.bfloat16, tag="h0")
                nc.scalar.activation(h0[0:H, :], ps0[0:H, 0:T], Relu,
                                     bias=b2c_s[:])
                nc.vector.memset(h0[H:H + 1, :], 1.0)

                # ---- out = h0.T @ W2aug (tree-major) ----
                osb = hpool.tile([128, 4 * H], dt.float32, tag="osb")
                for ch in range(4):
                    pso = pspool.tile([128, 2 * T], dt.float32, tag="g")
                    nc.tensor.matmul(pso[:, 0:H],
                                     h0[:, ch * 128:(ch + 1) * 128],
                                     w2aug_s[:], start=True, stop=True)
                    nc.vector.tensor_copy(out=osb[:, ch * H:(ch + 1) * H],
                                          in_=pso[:, 0:H])
                for ch in range(4):
                    nc.sync.dma_start(
                        out_d[it * T + ch * 128:it * T + (ch + 1) * 128, :],
                        osb[:, ch * H:(ch + 1) * H])

    split_waits(nc)
    return nc


def _make_in_maps(host):
    (ohA, ohB, tblA, tblB, W2W1, D, W2aug, b2c) = host
    in_maps = []
    for c in range(NCORES):
        s = slice(c * NPC, (c + 1) * NPC)
        a = ohA[:, :, s].reshape(RA, NODES, NT, T).transpose(0, 2, 1, 3)
        b = ohB[:, :, s].reshape(RB, 16, NT, T).transpose(0, 2, 1, 3)
        in_maps.append({
            "ohA": np.ascontiguousarray(a.reshape(RA, NT * NODES * T)),
            "ohB": np.ascontiguousarray(b.reshape(RB, NT * 16 * T)),
            "tblA": tblA, "tblB": tblB, "w2w1": W2W1, "dmat": D,
            "w2aug": W2aug, "b2c": b2c,
        })
    return in_maps


def kernel(**inputs) -> np.ndarray:
    from concourse.bass_utils import run_bass_kernel_spmd

    host = _build_host_tensors(**inputs)
    if "nc" not in _CACHED:
        _CACHED["nc"] = _build_program()
    nc = _CACHED["nc"]
    in_maps = _make_in_maps(host)
    res = run_bass_kernel_spmd(nc, in_maps, list(range(NCORES)))
    out = np.concatenate([res.results[c]["out"] for c in range(NCORES)],
                         axis=0)
    return out.astype(np.float32)
